# revision 6
# baseline (speedup 1.0000x reference)
"""MoE top-1 routing with expert capacity (nn_ExpertAllocation) on 8 TRN2 cores.

Strategy:
- Data-parallel over tokens: 16384 tokens -> 8 shards of 2048.
- Router GEMM: host splits x and W into bf16 hi/lo; device loads x^T via
  HW DMA-transpose (bf16) and runs a 4-pass bf16 matmul (hi/lo x hi/lo W),
  accumulated in fp32 PSUM -> fp32-quality logits at bf16 speed.
  The two W column-halves are packed into PE column groups (tile_position)
  so the 4 passes cost ~2 streaming passes.
- logits^T [64, T] are PE-transposed back to [T(part), 64] tiles for
  softmax/argmax (free-dim reductions).
- One-hot = (logit == rowmax), capacity cumsum over the token dim via
  triangular-ones matmuls + a serial per-tile offset chain; cross-core
  segment offsets via an AllGather of per-core expert counts; aux loss from
  all-gathered count/prob sums.
"""

import os
import numpy as np
import ml_dtypes

import concourse.bacc as bacc
import concourse.bass as bass
import concourse.mybir as mybir
import concourse.tile as tile
from concourse import bass_utils

F32 = mybir.dt.float32
BF16 = mybir.dt.bfloat16
F16 = mybir.dt.float16
SC = float(2.0 ** -12)          # scale of the fp16 low pieces
FP16_MIN_NORMAL = 6.103515625e-05
AX = mybir.AxisListType
OP = mybir.AluOpType
ACTF = mybir.ActivationFunctionType

B, S, D, E = 4, 4096, 2048, 64
NCORES = 8
TOK = B * S                 # 16384
TPC = TOK // NCORES         # 2048 tokens per core
CAP = float(TOK) / E * 1.0  # 256.0
ALPHA = 0.01
NJ = D // 128               # 16 contraction chunks
NT = TPC // 128             # 16 token tiles per core
NG = 4                      # token groups per core
GT = TPC // NG              # 512 tokens per group
TPG = GT // 128             # 4 token tiles per group


def build_program(single_core=False):
    """single_core=True replaces the collective with a local DMA so the
    program can run under single-core simulators (timing analysis only)."""
    nc = bacc.Bacc("TRN2", target_bir_lowering=False, debug=False,
                   enable_asserts=True,
                   num_devices=1 if single_core else NCORES)

    xh = nc.dram_tensor("xh", [TPC, D], F16, kind="ExternalInput").ap()
    xm = nc.dram_tensor("xm", [TPC, D], F16, kind="ExternalInput").ap()
    wH = nc.dram_tensor("wH", [D, E], F16, kind="ExternalInput").ap()
    wM = nc.dram_tensor("wM", [D, E], F16, kind="ExternalInput").ap()
    bias = nc.dram_tensor("bias", [1, E], F32, kind="ExternalInput").ap()
    triu = nc.dram_tensor("triu", [128, 128], BF16, kind="ExternalInput").ap()
    ident64 = nc.dram_tensor("ident64", [64, 64], F32, kind="ExternalInput").ap()
    onesrow = nc.dram_tensor("onesrow", [1, 128], F32, kind="ExternalInput").ap()
    onescol = nc.dram_tensor("onescol", [128, 1], F32, kind="ExternalInput").ap()
    prevmask = nc.dram_tensor("prevmask", [NCORES, 1], F32, kind="ExternalInput").ap()

    out = nc.dram_tensor("out", [TPC, E], F32, kind="ExternalOutput").ap()
    aux = nc.dram_tensor("aux", [1, 1], F32, kind="ExternalOutput").ap()

    cc_in = nc.dram_tensor("cc_in", [1, 2 * E], F32, kind="Internal")
    cc_out = nc.dram_tensor("cc_out", [NCORES, 2 * E], F32, kind="Internal")

    with tile.TileContext(nc) as tc:
        with tc.tile_pool(name="consts", bufs=1) as consts, \
             tc.tile_pool(name="xt", bufs=2) as xtp, \
             tc.tile_pool(name="work", bufs=3) as work, \
             tc.tile_pool(name="keep", bufs=NT) as keep, \
             tc.tile_pool(name="fin", bufs=1) as finp, \
             tc.tile_pool(name="plog", bufs=2, space="PSUM") as plog, \
             tc.tile_pool(name="psmall", bufs=2, space="PSUM") as psmall, \
             tc.tile_pool(name="pacc", bufs=2, space="PSUM") as pacc:

            # ---- constants ----
            wH_sb = consts.tile([128, NJ, E], F16)
            nc.sync.dma_start(wH_sb[:], wH.rearrange("(j p) e -> p j e", p=128))
            wM_sb = consts.tile([128, NJ, E], F16)
            nc.sync.dma_start(wM_sb[:], wM.rearrange("(j p) e -> p j e", p=128))
            triu_sb = consts.tile([128, 128], BF16)
            nc.sync.dma_start(triu_sb[:], triu)
            id64_sb = consts.tile([64, 64], F32)
            nc.sync.dma_start(id64_sb[:], ident64)
            ones_r = consts.tile([1, 128], F32)
            nc.sync.dma_start(ones_r[:], onesrow)
            ones_c = consts.tile([128, 1], F32)
            nc.sync.dma_start(ones_c[:], onescol)
            pmask = consts.tile([NCORES, 1], F32)
            nc.sync.dma_start(pmask[:], prevmask)
            b1 = consts.tile([1, E], F32)
            nc.sync.dma_start(b1[:], bias)
            bB = consts.tile([128, E], F32)
            nc.gpsimd.partition_broadcast(bB[:], b1[:])

            # running per-expert counts: slot t holds counts before tile t
            offs = consts.tile([1, (NT + 1) * E], F32)
            nc.vector.memset(offs[0:1, 0:E], 0.0)

            # P_i accumulator (sum of probs over this core's tokens)
            p_P = pacc.tile([1, E], F32, tag="pacc")

            ru_k = []   # routed probs (probs * onehot), kept per tile
            cum_k = []  # local inclusive cumsum counts, kept per tile

            NSPL = 4            # j-chunks per transposed load (overlap grain)
            JSP = NJ // NSPL    # 4 j per chunk
            for g in range(NG):
                # ---- transposed loads, split j-wise for DMA/PE overlap ----
                xth = []
                xtm = []
                for s in range(NSPL):
                    dsl = slice(s * JSP * 128, (s + 1) * JSP * 128)
                    th = xtp.tile([128, JSP, GT], F16, tag=f"xth{s}",
                                  name=f"xth{s}")
                    nc.sync.dma_start_transpose(
                        th[:], xh[g * GT:(g + 1) * GT, dsl])
                    xth.append(th)
                    tm = xtp.tile([128, JSP, GT], F16, tag=f"xtm{s}",
                                  name=f"xtm{s}")
                    nc.sync.dma_start_transpose(
                        tm[:], xm[g * GT:(g + 1) * GT, dsl])
                    xtm.append(tm)

                # ---- 3-term GEMM: logits = hH + SC*(hM + mH) ----
                # pA: rows 0:64 <- xh@wH, rows 64:128 <- xh@wM (col-packed)
                # pB: xm@wH
                pA = plog.tile([128, GT], F32, tag="pA")
                pB = plog.tile([64, GT], F32, tag="pB")
                for j in range(NJ):
                    st, sp = (j == 0), (j == NJ - 1)
                    s, jj = divmod(j, JSP)
                    nc.tensor.matmul(pA[0:64, :], wH_sb[:, j, :], xth[s][:, jj, :],
                                     start=st, stop=sp, tile_position=(0, 0))
                    nc.tensor.matmul(pA[64:128, :], wM_sb[:, j, :], xth[s][:, jj, :],
                                     start=st, stop=sp, tile_position=(0, 64))
                    nc.tensor.matmul(pB[:, :], wH_sb[:, j, :], xtm[s][:, jj, :],
                                     start=st, stop=sp)
                # combine: ltB = (copy(pA_hM) + pB)*SC + pA_hH
                ltA = work.tile([64, GT], F32, tag="ltA")
                nc.scalar.copy(ltA[:], pA[64:128, :])
                ltS = work.tile([64, GT], F32, tag="ltS")
                nc.vector.tensor_tensor(ltS[:], ltA[:], pB[:, :], op=OP.add)
                ltB = work.tile([64, GT], F32, tag="ltB")
                nc.vector.scalar_tensor_tensor(ltB[:], ltS[:], SC, pA[0:64, :],
                                               op0=OP.mult, op1=OP.add)

                for i in range(TPG):
                    t = g * TPG + i
                    sl = slice(i * 128, (i + 1) * 128)

                    # logits tile back to [128(tok), 64(e)]
                    p_lg = psmall.tile([128, E], F32, tag="psmall")
                    nc.tensor.transpose(p_lg[:], ltB[:, sl], id64_sb[:])

                    lg = work.tile([128, E], F32, tag="lg")
                    nc.vector.tensor_tensor(lg[:], p_lg[:], bB[:], op=OP.add)

                    # softmax pieces
                    m = work.tile([128, 1], F32, tag="m")
                    nc.vector.reduce_max(m[:], lg[:], axis=AX.X, negate=True)
                    ex = work.tile([128, E], F32, tag="ex")
                    ssum = work.tile([128, 1], F32, tag="ssum")
                    nc.scalar.activation(ex[:], lg[:], ACTF.Exp,
                                         bias=m[:], scale=1.0, accum_out=ssum[:])
                    rcp = work.tile([128, 1], F32, tag="rcp")
                    nc.vector.reciprocal(rcp[:], ssum[:])
                    probs = work.tile([128, E], F32, tag="probs")
                    nc.vector.tensor_scalar(probs[:], ex[:], rcp[:], None, op0=OP.mult)

                    # one-hot of argmax: (logit + (-max)) == 0
                    oh = work.tile([128, E], BF16, tag="oh")
                    nc.vector.tensor_scalar(oh[:], lg[:], m[:], 0.0,
                                            op0=OP.add, op1=OP.is_equal)

                    # routed prob = probs * onehot (kept for phase 3)
                    ru = keep.tile([128, E], F32, tag="ru")
                    nc.vector.tensor_tensor(ru[:], probs[:], oh[:], op=OP.mult)
                    ru_k.append(ru)

                    # P_i partial sums: ones^T @ probs accumulated over tiles
                    nc.tensor.matmul(p_P[:], ones_c[:], probs[:],
                                     start=(t == 0), stop=(t == NT - 1))

                    # local cumsum: triu^T (prefix) + broadcast of offs[t]
                    p_cum = psmall.tile([128, E], F32, tag="psmall")
                    nc.tensor.matmul(p_cum[:], triu_sb[:], oh[:],
                                     start=True, stop=False)
                    nc.tensor.matmul(p_cum[:], ones_r[:],
                                     offs[0:1, t * E:(t + 1) * E],
                                     start=False, stop=True)
                    # next tile's offset = offs[t] + per-expert counts of this
                    # tile (ones^T @ oh; triu's last column is all-ones)
                    p_cs = pacc.tile([1, E], F32, tag="pacc")
                    nc.tensor.matmul(p_cs[:], triu_sb[:, 127:128], oh[:],
                                     start=True, stop=True)
                    nc.vector.tensor_tensor(offs[0:1, (t + 1) * E:(t + 2) * E],
                                            offs[0:1, t * E:(t + 1) * E],
                                            p_cs[:], op=OP.add)
                    cum = keep.tile([128, E], F32, tag="cum")
                    nc.scalar.copy(cum[:], p_cum[:])
                    cum_k.append(cum)

            # ---- cross-core exchange: [counts | probsums] ----
            stats = work.tile([1, 2 * E], F32, tag="stats")
            nc.vector.tensor_copy(stats[0:1, 0:E], offs[0:1, NT * E:(NT + 1) * E])
            nc.vector.tensor_copy(stats[0:1, E:2 * E], p_P[:])
            nc.sync.dma_start(cc_in.ap(), stats[:])
            if single_core:
                nc.sync.dma_start(cc_out.ap()[0:1, :], cc_in.ap())
            else:
                nc.gpsimd.collective_compute(
                    "AllGather", OP.bypass,
                    replica_groups=[list(range(NCORES))],
                    ins=[cc_in.ap()], outs=[cc_out.ap()])
            gath = work.tile([NCORES, 2 * E], F32, tag="gath")
            nc.sync.dma_start(gath[:], cc_out.ap())

            # per-core segment offset = sum of previous cores' counts
            p_off = pacc.tile([1, 2 * E], F32, tag="pacc")
            nc.tensor.matmul(p_off[:], pmask[:], gath[:], start=True, stop=True)
            coreoff = work.tile([1, E], F32, tag="coreoff")
            nc.vector.tensor_copy(coreoff[:], p_off[0:1, 0:E])
            offB = work.tile([128, E], F32, tag="offB")
            nc.gpsimd.partition_broadcast(offB[:], coreoff[:])

            # aux loss = ALPHA*E * sum(counts/TOK * probsum/TOK)
            p_tot = pacc.tile([1, 2 * E], F32, tag="pacc")
            nc.tensor.matmul(p_tot[:], ones_c[0:NCORES, :], gath[:],
                             start=True, stop=True)
            tots = work.tile([1, 2 * E], F32, tag="tots")
            nc.vector.tensor_copy(tots[:], p_tot[:])
            fp = work.tile([1, E], F32, tag="fp")
            nc.vector.tensor_tensor(fp[:], tots[0:1, 0:E], tots[0:1, E:2 * E],
                                    op=OP.mult)
            auxv = work.tile([1, 1], F32, tag="auxv")
            nc.vector.reduce_sum(auxv[:], fp[:], axis=AX.X)
            aux_sb = work.tile([1, 1], F32, tag="aux_sb")
            nc.vector.tensor_scalar(aux_sb[:], auxv[:],
                                    float(ALPHA * E / (TOK * TOK)), None,
                                    op0=OP.mult)
            nc.sync.dma_start(aux, aux_sb[:])

            # ---- capacity mask + final output ----
            fin = finp.tile([128, NT, E], F32)
            out_r = out.rearrange("(t p) e -> p t e", p=128)
            for t in range(NT):
                tot = work.tile([128, E], F32, tag="tot")
                nc.vector.tensor_tensor(tot[:], cum_k[t][:], offB[:], op=OP.add)
                msk = work.tile([128, E], F32, tag="msk")
                nc.vector.tensor_scalar(msk[:], tot[:], CAP, None, op0=OP.is_le)
                nc.vector.tensor_tensor(fin[:, t, :], msk[:], ru_k[t][:],
                                        op=OP.mult)
                if t % 4 == 3:
                    nc.sync.dma_start(out_r[:, t - 3:t + 1, :],
                                      fin[:, t - 3:t + 1, :])

    nc.compile()
    return nc


_CACHE = {}


def _get_program():
    if "nc" not in _CACHE:
        _CACHE["nc"] = build_program()
    return _CACHE["nc"]


def _split_fp16(a):
    """a ~= ah + 2^-12 * am, both fp16, subnormals flushed host-side."""
    ah = a.astype(np.float16).astype(np.float32)
    ah[np.abs(ah) < FP16_MIN_NORMAL] = 0.0
    ah16 = ah.astype(np.float16)
    am = ((a - ah) * float(2.0 ** 12)).astype(np.float16).astype(np.float32)
    am[np.abs(am) < FP16_MIN_NORMAL] = 0.0
    return ah16, am.astype(np.float16)


def _prep_inputs(x, W, b):
    bf = ml_dtypes.bfloat16
    xf = np.ascontiguousarray(x.reshape(TOK, D)).astype(np.float32)
    xh, xm = _split_fp16(xf)
    Wf = np.asarray(W, dtype=np.float32)
    wH, wM = _split_fp16(Wf)
    bias = np.asarray(b, dtype=np.float32).reshape(1, E)
    triu = np.triu(np.ones((128, 128), dtype=np.float32)).astype(bf)
    ident64 = np.eye(64, dtype=np.float32)
    onesrow = np.ones((1, 128), dtype=np.float32)
    onescol = np.ones((128, 1), dtype=np.float32)

    in_maps = []
    for c in range(NCORES):
        pm = np.zeros((NCORES, 1), dtype=np.float32)
        pm[:c] = 1.0
        sl = slice(c * TPC, (c + 1) * TPC)
        in_maps.append({
            "xh": np.ascontiguousarray(xh[sl]),
            "xm": np.ascontiguousarray(xm[sl]),
            "wH": wH, "wM": wM, "bias": bias, "triu": triu,
            "ident64": ident64, "onesrow": onesrow, "onescol": onescol,
            "prevmask": pm,
        })
    return in_maps


def run(x, W, b, trace=False):
    nc = _get_program()
    in_maps = _prep_inputs(x, W, b)
    res = bass_utils.run_bass_kernel_spmd(
        nc, in_maps, core_ids=list(range(NCORES)), trace=trace)
    outs = np.concatenate([res.results[c]["out"] for c in range(NCORES)], axis=0)
    routed = outs.reshape(B, S, E).astype(np.float32)
    aux_loss = np.float32(res.results[0]["aux"][0, 0])
    return (routed, aux_loss), res


def kernel(x, W, b):
    (routed, aux_loss), _ = run(x, W, b, trace=False)
    return routed, aux_loss
